# revision 53
# baseline (speedup 1.0000x reference)
"""Multi-head attention forward on 8 Trainium2 NeuronCores.

Problem: nn_Attention_89060441850459
  inputs [8, 1024, 768] f32, w_qkv [768, 2304], w_proj [768, 768], b_proj [768]
  out = proj(softmax(q k^T / sqrt(64)) v) + b_proj,  H=12 heads, hd=64

Sharding: data parallel over batch - each of the 8 cores computes one batch
element end-to-end; weights replicated. No collectives.

v2 architecture (vs the serial-phase v1 at 224us):
  The kernel is jointly limited by the ACT engine (96 exps of [128,1024]
  ~ 107us) and the PE (f16 streaming total ~ 128us incl 2cyc/row f32
  transposes). v1 serialized lead (PE) -> attention (ACT) -> proj (PE).
  v2 overlaps them:
  - scalar engine runs ONLY exp (casts -> gpsimd, weight DMA issues ->
    sync queue, osb copies -> DVE/gpsimd, norm muls -> DVE/gpsimd).
  - attention chunk order is n2-OUTER (p, n2, m): only 2 oaug PSUM banks
    live instead of 4, freeing a dedicated 2-bank "qk" accumulator pool.
  - v/qkT projections are 6-matmul PSUM-accumulation chains; pair-0 +
    v0-3 run dense in the lead (~15us), the rest feed into the window's
    PE slack at ~2 matmuls/chunk via a work queue.
  - x is transposed in f32 (no cast needed), 6 transposes batched per
    [128,1024] PSUM tile, one strided DVE copy into a single xTall tile.
  - softmax Z comes free from a ones-column in the padded V (row 64 of
    the PV accumulator); normalization = reciprocal of the Z row +
    gpsimd partition_broadcast + one mul (no DRAM bounces).
  - proj drains on the freed mm PSUM slots in interleaved waves of 2.
"""

import sys

if "/opt/trn_rl_repo" not in sys.path:
    sys.path.insert(0, "/opt/trn_rl_repo")

from collections import deque
from contextlib import ExitStack

import numpy as np

import concourse.bass as bass
import concourse.mybir as mybir
import concourse.tile as tile
from concourse import bacc
from concourse.masks import make_identity

B, N, D = 8, 1024, 768
H = 12
HD = D // H  # 64
NCORES = 8
P = 128
NT = N // P  # 8 seq chunks
DC = D // P  # 6 d chunks
F32 = mybir.dt.float32
F16 = mybir.dt.float16
SCALE = HD**-0.5
HP = HD + 1  # head cols in padded v (64 v + 1 ones)


def build_attention(ctx: ExitStack, tc: "tile.TileContext", x, w_qkv, w_proj, b_proj, y):
    nc = tc.nc
    exp = mybir.ActivationFunctionType.Exp

    perm = ctx.enter_context(tc.tile_pool(name="perm", bufs=1))
    mm = ctx.enter_context(tc.tile_pool(name="mmps", bufs=2, space="PSUM"))
    attps = ctx.enter_context(tc.tile_pool(name="attps", bufs=2, space="PSUM"))
    qkp = ctx.enter_context(tc.tile_pool(name="qkps", bufs=2, space="PSUM"))
    att = ctx.enter_context(tc.tile_pool(name="att", bufs=2))
    stage = ctx.enter_context(tc.tile_pool(name="stage", bufs=2))
    zspill = ctx.enter_context(tc.tile_pool(name="zspill", bufs=2, space="DRAM"))

    # identity first: the PE warm-up below depends on it
    identity = perm.tile([P, P], F32, tag="identity", name="identity")
    make_identity(nc, identity)
    wps = qkp.tile([P, N // 2], F32, tag="qk", name="warmps")
    for _ in range(12):
        nc.tensor.matmul(
            wps[:, 0:P],
            lhsT=identity,
            rhs=identity,
            is_transpose=True,
            start=True,
            stop=True,
            skip_group_check=True,
        )

    # persistent SBUF arrays
    xTall = perm.tile([P, DC * N], F16, tag="xTall", name="xTall")  # [d| k*1024+n]
    qkT = [perm.tile([P, N], F16, tag=f"qkT{m}", name=f"qkT{m}") for m in range(12)]
    vpad = [perm.tile([P, H * HP], F16, tag=f"vpad{i}", name=f"vpad{i}") for i in range(NT)]
    oT = [perm.tile([P, N], F16, tag=f"oT{j}", name=f"oT{j}") for j in range(DC)]
    wq = [perm.tile([P, 3 * D], F16, tag=f"wq{k}", name=f"wq{k}") for k in range(DC)]
    wp = [perm.tile([P, D], F16, tag=f"wp{k}", name=f"wp{k}") for k in range(DC)]
    brep = perm.tile([P, D], F32, tag="brep", name="brep")

    # ---------------- input DMAs + casts ----------------
    # ones columns of padded v (value-independent: memset once up front)
    for i in range(NT):
        vp3 = vpad[i].rearrange("p (h c) -> p h c", c=HP)
        nc.gpsimd.memset(vp3[:, :, HD : HD + 1], 1.0)

    # x chunks: [128, 768] f32; first 4 before v-col weights (transposes gate
    # everything), remaining interleaved so the first qkT/v chains aren't
    # starved behind the full 3MB of x.
    # DMA issue order = HBM arrival order: x first (transposes gate all),
    # then the small pair-0 q/k column slices (0.8MB - land fast so the
    # window can start ~20us), then v cols, then the bulk q/k + w_proj.
    xt = [stage.tile([P, D], F32, tag="x", name=f"xt{i}", bufs=2) for i in range(NT)]
    w32qk0 = [stage.tile([P, 2 * P], F32, tag="w32qk0", name=f"w32qk0{k}", bufs=6) for k in range(DC)]
    w32v = [stage.tile([P, D], F32, tag="w32v", name=f"w32v{k}", bufs=2) for k in range(DC)]
    # rest of the q/k columns: all 6 resident so the bulk DMA stream never
    # waits on the window-fed casts that consume it
    w32qkr = [stage.tile([P, 2 * 640], F32, tag="w32qkr", name=f"w32qkr{k}", bufs=6) for k in range(DC)]
    w32p = [stage.tile([P, D], F32, tag="w32p", name=f"w32p{k}", bufs=2) for k in range(DC)]

    # Ring discipline: each engine queue owns one DMA ring that processes
    # its DMAs IN ORDER with ~1.5us turnaround per transfer, so small
    # weight slices between x chunks starve the transposes. sync ring =
    # x only; the pair-0 slices split across the idle tensor and gpsimd
    # rings; bulk follows on those rings by first-use time.
    for i in range(NT):
        nc.sync.dma_start(out=xt[i], in_=x[i * P : (i + 1) * P, :])
    nc.sync.dma_start(out=brep, in_=b_proj.partition_broadcast(P))
    for k in range(0, DC, 2):  # scalar ring: qk0 even k (idle until exp(0))
        nc.scalar.dma_start(out=w32qk0[k][:, 0:P], in_=w_qkv[k * P : (k + 1) * P, 0:P])
        nc.scalar.dma_start(out=w32qk0[k][:, P : 2 * P], in_=w_qkv[k * P : (k + 1) * P, D : D + P])
    for k in range(1, DC, 2):  # gpsimd ring: qk0 odd k
        nc.gpsimd.dma_start(out=w32qk0[k][:, 0:P], in_=w_qkv[k * P : (k + 1) * P, 0:P])
        nc.gpsimd.dma_start(out=w32qk0[k][:, P : 2 * P], in_=w_qkv[k * P : (k + 1) * P, D : D + P])
    nc.gpsimd.dma_start(out=w32v[0], in_=w_qkv[0:P, 2 * D : 3 * D])

    # pair-0 q/k casts on gpsimd (parallel with DVE's xTall copies)
    for k in range(DC):
        nc.gpsimd.tensor_copy(wq[k][:, 0:P], w32qk0[k][:, 0:P])
        nc.gpsimd.tensor_copy(wq[k][:, D : D + P], w32qk0[k][:, P : 2 * P])

    # bulk: q/k rest columns k0-2 + v cols on the gpsimd ring, k3-5 on the
    # tensor ring (emitted after the dense lead), w_proj last
    for k in range(3):
        nc.gpsimd.dma_start(out=w32qkr[k][:, 0:640], in_=w_qkv[k * P : (k + 1) * P, P:D])
        nc.gpsimd.dma_start(out=w32qkr[k][:, 640:1280], in_=w_qkv[k * P : (k + 1) * P, D + P : 2 * D])
    for k in range(1, DC):
        nc.gpsimd.dma_start(out=w32v[k], in_=w_qkv[k * P : (k + 1) * P, 2 * D : 3 * D])
    # v-col casts k1-5 (gpsimd stalls on late DMAs are harmless there)
    for k in range(1, DC):
        nc.gpsimd.tensor_copy(wq[k][:, 2 * D : 3 * D], w32v[k])
    for k in range(DC):
        nc.gpsimd.dma_start(out=w32p[k], in_=w_proj[k * P : (k + 1) * P, :])

    # ---------------- transposes: xTall[d, k*1024 + n] = x[n, d] ----------------
    # f32 transposes (2 cyc/row) straight from the DMA'd x - no cast pass.
    # 6 per [128,1024] mm PSUM tile, one strided DVE copy (cast to f16).
    # NOTE: a start=True matmul marks its full 2KB PSUM region (512 f32 cols)
    # pending-zero, so only the first transpose per bank may set start; the
    # rest accumulate onto pending-zero bytes (reads as 0).
    # DVE queue order matters: xTall copies FIRST (they gate all chains),
    # then v-col casts, then pair-0 q/k casts. The rest of the casts go
    # into the work queue (DVE would otherwise sit on weight DMAs while
    # the transposed x waits).
    def emit_transpose(i):
        pt = mm.tile([P, N], F32, tag="mm", name="tps")
        for j in range(DC):
            nc.tensor.matmul(
                pt[:, j * P : (j + 1) * P],
                lhsT=xt[i][:, j * P : (j + 1) * P],
                rhs=identity,
                is_transpose=True,
                start=(j % 4 == 0),
                stop=(j % 4 == 3 or j == DC - 1),
                skip_group_check=True,
            )
        nc.vector.tensor_copy(
            xTall.rearrange("p (k n) -> p k n", n=N)[:, :, i * P : (i + 1) * P],
            pt[:, 0 : DC * P].rearrange("p (k c) -> p k c", c=P),
        )

    for i in range(4):
        emit_transpose(i)
    nc.vector.tensor_copy(wq[0][:, 2 * D : 3 * D], w32v[0])
    for i in range(4, NT):
        emit_transpose(i)

    def xT(k):
        return xTall[:, k * N : (k + 1) * N]

    # ---------------- matmul chain generators ----------------
    # yields ("mm", thunk) for PE matmuls and ("other", thunk) for the
    # finishing copy; chains accumulate 6 k-steps in a qkp PSUM tile.
    def v_chain(m, c0, cw):
        # uniform [128,512] qkp tiles (bank-exclusive) even for the 256-col
        # chain, so start=True zero-regions never alias a neighbor
        ps_full = qkp.tile([P, N // 2], F32, tag="qk", name="vps")
        ps = ps_full[:, 0:cw]
        for k in range(DC):

            def job(k=k, ps=ps):
                nc.tensor.matmul(
                    ps,
                    lhsT=xT(k)[:, m * P : (m + 1) * P],
                    rhs=wq[k][:, 2 * D + c0 : 2 * D + c0 + cw],
                    start=(k == 0),
                    stop=(k == DC - 1),
                    skip_group_check=True,
                )

            yield ("mm", job)

        def finish(ps=ps):
            vp3 = vpad[m].rearrange("p (h c) -> p h c", c=HP)
            h0 = c0 // HD
            nc.vector.tensor_copy(
                vp3[:, h0 : h0 + cw // HD, 0:HD],
                ps.rearrange("p (h c) -> p h c", c=HD),
            )

        yield ("other", finish)

    def qkT_chain(t, half):
        ps = qkp.tile([P, N // 2], F32, tag="qk", name="qkps")
        for k in range(DC):

            def job(k=k, ps=ps):
                nc.tensor.matmul(
                    ps,
                    lhsT=wq[k][:, t * P : (t + 1) * P],
                    rhs=xT(k)[:, half * 512 : (half + 1) * 512],
                    start=(k == 0),
                    stop=(k == DC - 1),
                    skip_group_check=True,
                )

            yield ("mm", job)

        def finish(ps=ps):
            nc.vector.tensor_copy(qkT[t][:, half * 512 : (half + 1) * 512], ps)

        yield ("other", finish)

    # dense lead: pair-0 q/k half-0 (gates S(0)/exp(0) - the window start)
    # and v0 (PV chunk 0); v1-3 ride the queue thanks to the PV lag.
    for t in (6, 0):
        for kind, job in qkT_chain(t, 0):
            job()
    for c0, cw in ((0, 512), (512, 256)):
        for kind, job in v_chain(0, c0, cw):
            job()
    # bulk q/k rest columns k3-5 on the sync ring (free after x)
    for k in range(3, DC):
        nc.sync.dma_start(out=w32qkr[k][:, 0:640], in_=w_qkv[k * P : (k + 1) * P, P:D])
        nc.sync.dma_start(out=w32qkr[k][:, 640:1280], in_=w_qkv[k * P : (k + 1) * P, D + P : 2 * D])

    # work queue: (deadline_chunk, kind, thunk). Tile semantics are
    # sequential-program-order, so a chain MUST be emitted before the
    # chunk that consumes it: feed(t) drains every item with deadline
    # <= t+2 (covers the s(t+2)/o(t+1) emitted in iteration t+1), and
    # additionally up to `quota` matmuls per chunk to spread the rest
    # evenly through the window's PE slack.
    work = deque()
    mm_left = 0

    def push(deadline, gen):
        nonlocal mm_left
        for kind, job in gen:
            work.append((deadline, kind, job))
            if kind == "mm":
                mm_left += 1

    # PV emission lags exp/S by LAG chunks, so v-chain deadlines sit past
    # the S-critical qkT chains and the quota can spread them (deadline
    # bursts of v chains ahead of S supply starved exp for ~16us).
    LAG = 8

    def cast_job(dst, src):
        yield ("other", lambda: nc.vector.tensor_copy(dst, src))

    push(4, qkT_chain(6, 1))  # S(0,0,m>=4) reads k-tile cols 512:1024
    push(8, qkT_chain(0, 1))  # S(0,1,*) rhs
    for m in (1, 2):
        for c0, cw in ((0, 512), (512, 256)):
            push(m + LAG, v_chain(m, c0, cw))
    # deferred weight casts (DVE): q/k tiles 1-5,7-11 cols
    for k in range(DC):
        dl = 10 + (k // 2)
        push(dl, cast_job(wq[k][:, P:D], w32qkr[k][:, 0:640]))
        push(dl, cast_job(wq[k][:, D + P : 2 * D], w32qkr[k][:, 640:1280]))
    for m in (3, 4, 5, 6, 7):
        for c0, cw in ((0, 512), (512, 256)):
            push(m + LAG, v_chain(m, c0, cw))
    # keep deadlines monotone: the drain in feed() only inspects the head
    for p in range(1, 6):
        push(16 * p, qkT_chain(6 + p, 0))
        push(16 * p, qkT_chain(p, 0))
        push(16 * p + 4, qkT_chain(6 + p, 1))
        push(16 * p + 8, qkT_chain(p, 1))
    for k in range(DC):
        push(88, cast_job(wp[k], w32p[k]))

    def feed(t, T):
        nonlocal mm_left
        rem = T - t
        quota = min(3, -(-mm_left // rem)) if rem > 0 else mm_left
        issued = 0
        while work:
            deadline, kind, job = work[0]
            if deadline > t + 2 and issued >= quota:
                break
            work.popleft()
            job()
            if kind == "mm":
                issued += 1
                mm_left -= 1

    # ---------------- attention window ----------------
    # chunks (p, n2, m): n2-OUTER so only one qpos-half's oaug pair is live.
    chunks = [(p, n2, m) for p in range(H // 2) for n2 in range(2) for m in range(NT)]
    T = len(chunks)
    sps = {}
    epool = {}
    oaug = {}
    osbs = {}

    def emit_s(t):
        p, n2, m = chunks[t]
        sp = mm.tile([P, N], F32, tag="mm", name="sps")
        sps[t] = sp
        for half in range(2):
            row = half * HD
            nc.tensor.matmul(
                sp[:, half * 512 : (half + 1) * 512],
                lhsT=qkT[6 + p][row : row + HD, m * P : (m + 1) * P],
                rhs=qkT[p][row : row + HD, n2 * 512 : (n2 + 1) * 512],
                start=True,
                stop=True,
            )

    def emit_exp(t):
        e = att.tile([P, N], F16, tag="e", name="etile", bufs=11)
        epool[t] = e
        nc.scalar.activation(e, sps.pop(t), exp, scale=SCALE)

    def emit_osb(h, n2, oa):
        # gpsimd cannot read PSUM; copies go to DVE (emitted ahead of the
        # feed so they run early and release the oaug slots promptly).
        # Both qpos halves of a head land in ONE [65,1024] tile so the
        # norm runs once per head.
        if n2 == 0:
            osbs[h] = att.tile([HP, N], F32, tag="osb", name="osb", bufs=3)
        nc.vector.tensor_copy(osbs[h][:, n2 * 512 : (n2 + 1) * 512], oa)

    z8s = {}

    def emit_zfetch(h, n2, oa):
        # partition-spread the Z row [1,512] -> [128,4]: bounce through
        # DRAM (only DRAM APs can remap free elements onto partitions);
        # launched at each half's end so it's off the tail's critical path
        if n2 == 0:
            z8s[h] = att.tile([P, 8], F32, tag="z8", name="z8", bufs=3)
        zd = zspill.tile([1, N // 2], F32, tag="zd", name="zd", bufs=3)
        nc.sync.dma_start(out=zd, in_=osbs[h][HD : HD + 1, n2 * 512 : (n2 + 1) * 512])
        nc.sync.dma_start(
            out=z8s[h][:, n2 * 4 : (n2 + 1) * 4],
            in_=zd.rearrange("o (p f) -> (o p) f", p=P),
        )

    def emit_norm(h, n2=None):
        # oT[h//2][row:row+64, cols] = osb rows 0:64 / Z  (Z = osb row 64).
        # reciprocal is ~6 cyc/ELEMENT serial per partition, so it runs on
        # the [128,4/8] partition-spread Z; 1/Z bounces through DRAM for
        # the partition-broadcast. n2=None: whole head (fewer DMAs);
        # n2 given: per qpos half (last pair, so only half-1 hits the tail).
        row = (h % 2) * HD
        if n2 is None:
            oX = osbs.pop(h)
            z8 = z8s.pop(h)
            r8 = att.tile([P, 8], F32, tag="r8", name="r8", bufs=2)
            nc.vector.reciprocal(r8, z8)
            rd = zspill.tile([1, N], F32, tag="rd", name="rd", bufs=2)
            nc.sync.dma_start(
                out=rd.rearrange("o (h p f) -> (o p) h f", h=2, f=4),
                in_=r8.rearrange("p (h f) -> p h f", f=4),
            )
            zb = att.tile([HD, N], F32, tag="zb", name="zbcast", bufs=2)
            nc.sync.dma_start(out=zb, in_=rd[0, :].partition_broadcast(HD))
            dst = oT[h // 2][row : row + HD, :]
            if h % 2 == 0:
                nc.vector.tensor_mul(dst, oX[0:HD, :], zb)
            else:
                nc.gpsimd.tensor_mul(dst, oX[0:HD, :], zb)
        else:
            oX = osbs[h] if n2 == 0 else osbs.pop(h)
            z8 = z8s[h] if n2 == 0 else z8s.pop(h)
            cols = slice(n2 * 512, (n2 + 1) * 512)
            r8 = att.tile([P, 8], F32, tag="r8", name="r8", bufs=2)[:, 0:4]
            nc.vector.reciprocal(r8, z8[:, n2 * 4 : (n2 + 1) * 4])
            rd = zspill.tile([1, N], F32, tag="rd", name="rd", bufs=2)[:, 0:512]
            nc.sync.dma_start(out=rd.rearrange("o (p f) -> (o p) f", p=P), in_=r8)
            zb = att.tile([HD, N], F32, tag="zb", name="zbcast", bufs=2)[:, 0:512]
            nc.sync.dma_start(out=zb, in_=rd[0, :].partition_broadcast(HD))
            dst = oT[h // 2][row : row + HD, cols]
            nc.vector.tensor_mul(dst, oX[0:HD, cols], zb)

    def emit_o(t):
        p, n2, m = chunks[t]
        e = epool.pop(t)
        for half in range(2):
            h = 2 * p + half
            if m == 0:
                oaug[h] = attps.tile([HP, N // 2], F32, tag="oaug", name="oaug", bufs=2)
            nc.tensor.matmul(
                oaug[h],
                lhsT=vpad[m][:, h * HP : (h + 1) * HP],
                rhs=e[:, half * 512 : (half + 1) * 512],
                start=(m == 0),
                stop=(m == NT - 1),
                skip_group_check=True,
            )
        if m == NT - 1:
            for half in range(2):
                h = 2 * p + half
                oa = oaug.pop(h)
                emit_osb(h, n2, oa)
                emit_zfetch(h, n2, oa)
            if p == H // 2 - 1:
                # last pair: per-half norms so only half-1 sits on the tail
                emit_norm(2 * p, n2)
                emit_norm(2 * p + 1, n2)
            elif n2 == 1:
                emit_norm(2 * p)
                emit_norm(2 * p + 1)

    emit_s(0)
    for t in range(T):
        emit_exp(t)
        if t + 1 < T:
            emit_s(t + 1)
        if t >= LAG:
            emit_o(t - LAG)
        feed(t, T)
    for t in range(T - LAG, T):
        emit_o(t)

    while work:
        _, _, job = work.popleft()
        job()

    # ---------------- proj drain: y = oT.T @ w_proj + b ----------------
    # 4 i-blocks in flight per round (2 full-width mm slots + col-split
    # chains on the freed qkp and attps slots), k-steps round-robin so the
    # k=5 steps (waiting on the last pair's norm) sit behind useful work.
    def proj_round(iblocks):
        pss = {}
        for g, i in enumerate(iblocks):
            if g < 2:
                ps = mm.tile([P, N], F32, tag="mm", name="pjps")
                pss[i] = (ps[:, 0:512], ps[:, 512:D], "mm")
            else:
                pool = qkp if g == 2 else attps
                tag = "qk" if g == 2 else "oaug"
                psA = pool.tile([P, 512], F32, tag=tag, name="pjA")
                psB = pool.tile([P, 512], F32, tag=tag, name="pjB")
                pss[i] = (psA, psB[:, 0:256], "cols")
        for k in range(DC):
            for i in iblocks:
                psA, psB, _ = pss[i]
                for ps_, c0, cw in ((psA, 0, 512), (psB, 512, 256)):
                    nc.tensor.matmul(
                        ps_,
                        lhsT=oT[k][:, i * P : (i + 1) * P],
                        rhs=wp[k][:, c0 : c0 + cw],
                        start=(k == 0),
                        stop=(k == DC - 1),
                        skip_group_check=True,
                    )
        for i in iblocks:
            psA, psB, kind = pss[i]
            yt = att.tile([P, D], F32, tag="yt", name="ytile", bufs=3)
            if kind == "mm":
                nc.vector.tensor_add(yt, psA.tensor[0:P, 0:D], brep)
            else:
                nc.vector.tensor_add(yt[:, 0:512], psA, brep[:, 0:512])
                nc.vector.tensor_add(yt[:, 512:D], psB, brep[:, 512:D])
            nc.sync.dma_start(out=y[i * P : (i + 1) * P, :], in_=yt)

    proj_round([0, 1, 2, 3])
    proj_round([4, 5, 6, 7])


def build_nc(debug: bool = False):
    nc = bacc.Bacc("TRN2", target_bir_lowering=False, debug=debug, enable_asserts=False)
    x = nc.dram_tensor("x", [N, D], F32, kind="ExternalInput").ap()
    w_qkv = nc.dram_tensor("w_qkv", [D, 3 * D], F32, kind="ExternalInput").ap()
    w_proj = nc.dram_tensor("w_proj", [D, D], F32, kind="ExternalInput").ap()
    b_proj = nc.dram_tensor("b_proj", [D], F32, kind="ExternalInput").ap()
    y = nc.dram_tensor("y", [N, D], F32, kind="ExternalOutput").ap()
    with tile.TileContext(nc) as tc:
        with ExitStack() as ctx:
            build_attention(ctx, tc, x, w_qkv, w_proj, b_proj, y)
    nc.compile()
    return nc


_NC = None


def _get_nc():
    global _NC
    if _NC is None:
        _NC = build_nc()
    return _NC


def kernel(inputs, w_qkv, w_proj, b_proj, _trace=False, **run_kwargs):
    from concourse.bass_utils import run_bass_kernel_spmd

    nc = _get_nc()
    inputs = np.asarray(inputs, dtype=np.float32)
    w_qkv = np.ascontiguousarray(np.asarray(w_qkv, dtype=np.float32))
    w_proj = np.ascontiguousarray(np.asarray(w_proj, dtype=np.float32))
    b_proj = np.ascontiguousarray(np.asarray(b_proj, dtype=np.float32))
    in_maps = [
        {
            "x": np.ascontiguousarray(inputs[i]),
            "w_qkv": w_qkv,
            "w_proj": w_proj,
            "b_proj": b_proj,
        }
        for i in range(NCORES)
    ]
    res = run_bass_kernel_spmd(nc, in_maps, list(range(NCORES)), trace=_trace, **run_kwargs)
    out = np.stack([res.results[i]["y"] for i in range(NCORES)], axis=0)
    if _trace:
        return out, res
    return out


# revision 61
# speedup vs baseline: 1.0517x; 1.0517x over previous
"""Multi-head attention forward on 8 Trainium2 NeuronCores.

Problem: nn_Attention_89060441850459
  inputs [8, 1024, 768] f32, w_qkv [768, 2304], w_proj [768, 768], b_proj [768]
  out = proj(softmax(q k^T / sqrt(64)) v) + b_proj,  H=12 heads, hd=64

Sharding: data parallel over batch - each of the 8 cores computes one batch
element end-to-end; weights replicated. No collectives.

v2 architecture (vs the serial-phase v1 at 224us):
  The kernel is jointly limited by the ACT engine (96 exps of [128,1024]
  ~ 107us) and the PE (f16 streaming total ~ 128us incl 2cyc/row f32
  transposes). v1 serialized lead (PE) -> attention (ACT) -> proj (PE).
  v2 overlaps them:
  - scalar engine runs ONLY exp (casts -> gpsimd, weight DMA issues ->
    sync queue, osb copies -> DVE/gpsimd, norm muls -> DVE/gpsimd).
  - attention chunk order is n2-OUTER (p, n2, m): only 2 oaug PSUM banks
    live instead of 4, freeing a dedicated 2-bank "qk" accumulator pool.
  - v/qkT projections are 6-matmul PSUM-accumulation chains; pair-0 +
    v0-3 run dense in the lead (~15us), the rest feed into the window's
    PE slack at ~2 matmuls/chunk via a work queue.
  - x is transposed in f32 (no cast needed), 6 transposes batched per
    [128,1024] PSUM tile, one strided DVE copy into a single xTall tile.
  - softmax Z comes free from a ones-column in the padded V (row 64 of
    the PV accumulator); normalization = reciprocal of the Z row +
    gpsimd partition_broadcast + one mul (no DRAM bounces).
  - proj drains on the freed mm PSUM slots in interleaved waves of 2.
"""

import sys

if "/opt/trn_rl_repo" not in sys.path:
    sys.path.insert(0, "/opt/trn_rl_repo")

from collections import deque
from contextlib import ExitStack

import numpy as np

import concourse.bass as bass
import concourse.mybir as mybir
import concourse.tile as tile
from concourse import bacc
from concourse.masks import make_identity

B, N, D = 8, 1024, 768
H = 12
HD = D // H  # 64
NCORES = 8
P = 128
NT = N // P  # 8 seq chunks
DC = D // P  # 6 d chunks
F32 = mybir.dt.float32
F16 = mybir.dt.float16
SCALE = HD**-0.5
HP = HD + 1  # head cols in padded v (64 v + 1 ones)


def build_attention(ctx: ExitStack, tc: "tile.TileContext", x, w_qkv, w_proj, b_proj, y):
    nc = tc.nc
    exp = mybir.ActivationFunctionType.Exp

    perm = ctx.enter_context(tc.tile_pool(name="perm", bufs=1))
    mm = ctx.enter_context(tc.tile_pool(name="mmps", bufs=2, space="PSUM"))
    attps = ctx.enter_context(tc.tile_pool(name="attps", bufs=2, space="PSUM"))
    qkp = ctx.enter_context(tc.tile_pool(name="qkps", bufs=2, space="PSUM"))
    att = ctx.enter_context(tc.tile_pool(name="att", bufs=2))
    stage = ctx.enter_context(tc.tile_pool(name="stage", bufs=2))
    zspill = ctx.enter_context(tc.tile_pool(name="zspill", bufs=2, space="DRAM"))

    # identity first: the PE warm-up below depends on it
    identity = perm.tile([P, P], F16, tag="identity", name="identity")
    make_identity(nc, identity)
    wps = qkp.tile([P, N // 2], F32, tag="qk", name="warmps")
    for _ in range(12):
        nc.tensor.matmul(
            wps.bitcast(F16)[:, 0:P],
            lhsT=identity,
            rhs=identity,
            is_transpose=True,
            start=True,
            stop=True,
            skip_group_check=True,
        )

    # persistent SBUF arrays
    xTall = perm.tile([P, DC * N], F16, tag="xTall", name="xTall")  # [d| k*1024+n]
    qkT = [perm.tile([P, N], F16, tag=f"qkT{m}", name=f"qkT{m}") for m in range(12)]
    vpad = [perm.tile([P, H * HP], F16, tag=f"vpad{i}", name=f"vpad{i}") for i in range(NT)]
    oT = [perm.tile([P, N], F16, tag=f"oT{j}", name=f"oT{j}") for j in range(DC)]
    wq = [perm.tile([P, 3 * D], F16, tag=f"wq{k}", name=f"wq{k}") for k in range(DC)]
    wp = [perm.tile([P, D], F16, tag=f"wp{k}", name=f"wp{k}") for k in range(DC)]
    brep = perm.tile([P, D], F32, tag="brep", name="brep")

    # ---------------- input DMAs + casts ----------------
    # ones columns of padded v (value-independent: memset once up front)
    for i in range(NT):
        vp3 = vpad[i].rearrange("p (h c) -> p h c", c=HP)
        nc.gpsimd.memset(vp3[:, :, HD : HD + 1], 1.0)

    # x chunks: [128, 768] f32; first 4 before v-col weights (transposes gate
    # everything), remaining interleaved so the first qkT/v chains aren't
    # starved behind the full 3MB of x.
    # DMA issue order = HBM arrival order: x first (transposes gate all),
    # then the small pair-0 q/k column slices (0.8MB - land fast so the
    # window can start ~20us), then v cols, then the bulk q/k + w_proj.
    xt = [stage.tile([P, D], F32, tag="x", name=f"xt{i}", bufs=2) for i in range(NT)]
    w32qk0 = [stage.tile([P, 2 * P], F32, tag="w32qk0", name=f"w32qk0{k}", bufs=6) for k in range(DC)]
    w32v = [stage.tile([P, D], F32, tag="w32v", name=f"w32v{k}", bufs=2) for k in range(DC)]
    # rest of the q/k columns: all 6 resident so the bulk DMA stream never
    # waits on the window-fed casts that consume it
    w32qkr = [stage.tile([P, 2 * 640], F32, tag="w32qkr", name=f"w32qkr{k}", bufs=6) for k in range(DC)]
    w32p = [stage.tile([P, D], F32, tag="w32p", name=f"w32p{k}", bufs=1) for k in range(DC)]

    # Ring discipline: each engine queue owns one DMA ring that processes
    # its DMAs IN ORDER with ~1.5us turnaround per transfer, so small
    # weight slices between x chunks starve the transposes. sync ring =
    # x only; the pair-0 slices split across the idle tensor and gpsimd
    # rings; bulk follows on those rings by first-use time.
    for i in range(NT):
        nc.sync.dma_start(out=xt[i], in_=x[i * P : (i + 1) * P, :])
    nc.sync.dma_start(out=brep, in_=b_proj.partition_broadcast(P))
    for k in range(0, DC, 2):  # scalar ring: qk0 even k (idle until exp(0))
        nc.scalar.dma_start(out=w32qk0[k][:, 0:P], in_=w_qkv[k * P : (k + 1) * P, 0:P])
        nc.scalar.dma_start(out=w32qk0[k][:, P : 2 * P], in_=w_qkv[k * P : (k + 1) * P, D : D + P])
    for k in range(1, DC, 2):  # gpsimd ring: qk0 odd k
        nc.gpsimd.dma_start(out=w32qk0[k][:, 0:P], in_=w_qkv[k * P : (k + 1) * P, 0:P])
        nc.gpsimd.dma_start(out=w32qk0[k][:, P : 2 * P], in_=w_qkv[k * P : (k + 1) * P, D : D + P])
    nc.gpsimd.dma_start(out=w32v[0], in_=w_qkv[0:P, 2 * D : 3 * D])

    # pair-0 q/k casts on gpsimd (parallel with DVE's xTall copies)
    for k in range(DC):
        nc.gpsimd.tensor_copy(wq[k][:, 0:P], w32qk0[k][:, 0:P])
        nc.gpsimd.tensor_copy(wq[k][:, D : D + P], w32qk0[k][:, P : 2 * P])

    # bulk: q/k rest columns k0-2 + v cols on the gpsimd ring, k3-5 on the
    # tensor ring (emitted after the dense lead), w_proj last
    for k in range(3):
        nc.gpsimd.dma_start(out=w32qkr[k][:, 0:640], in_=w_qkv[k * P : (k + 1) * P, P:D])
        nc.gpsimd.dma_start(out=w32qkr[k][:, 640:1280], in_=w_qkv[k * P : (k + 1) * P, D + P : 2 * D])
    for k in range(1, DC):
        nc.gpsimd.dma_start(out=w32v[k], in_=w_qkv[k * P : (k + 1) * P, 2 * D : 3 * D])
    # v-col casts k1-5 (gpsimd stalls on late DMAs are harmless there)
    for k in range(1, DC):
        nc.gpsimd.tensor_copy(wq[k][:, 2 * D : 3 * D], w32v[k])
    for k in range(DC):
        nc.gpsimd.dma_start(out=w32p[k], in_=w_proj[k * P : (k + 1) * P, :])

    # ---------------- transposes: xTall[d, k*1024 + n] = x[n, d] ----------------
    # f32 transposes (2 cyc/row) straight from the DMA'd x - no cast pass.
    # 6 per [128,1024] mm PSUM tile, one strided DVE copy (cast to f16).
    # NOTE: a start=True matmul marks its full 2KB PSUM region (512 f32 cols)
    # pending-zero, so only the first transpose per bank may set start; the
    # rest accumulate onto pending-zero bytes (reads as 0).
    # DVE queue order matters: xTall copies FIRST (they gate all chains),
    # then v-col casts, then pair-0 q/k casts. The rest of the casts go
    # into the work queue (DVE would otherwise sit on weight DMAs while
    # the transposed x waits).
    def emit_transpose(i):
        # f16 transposes (1 cyc/row vs f32's 2): DVE casts the x chunk
        # first, pipelined ahead of this chunk's xTall copy
        xc = stage.tile([P, D], F16, tag="x16", name=f"xc{i}", bufs=2)
        nc.vector.tensor_copy(xc, xt[i])
        pt = mm.tile([P, N], F32, tag="mm", name="tps")
        pt16 = pt.bitcast(F16)
        for j in range(DC):
            nc.tensor.matmul(
                pt16[:, j * P : (j + 1) * P],
                lhsT=xc[:, j * P : (j + 1) * P],
                rhs=identity,
                is_transpose=True,
                start=(j % 8 == 0),
                stop=(j % 8 == 7 or j == DC - 1),
                skip_group_check=True,
            )
        nc.vector.tensor_copy(
            xTall.rearrange("p (k n) -> p k n", n=N)[:, :, i * P : (i + 1) * P],
            pt16[:, 0 : DC * P].rearrange("p (k c) -> p k c", c=P),
        )

    for i in range(4):
        emit_transpose(i)
    nc.vector.tensor_copy(wq[0][:, 2 * D : 3 * D], w32v[0])
    for i in range(4, NT):
        emit_transpose(i)

    def xT(k):
        return xTall[:, k * N : (k + 1) * N]

    # ---------------- matmul chain generators ----------------
    # yields ("mm", thunk) for PE matmuls and ("other", thunk) for the
    # finishing copy; chains accumulate 6 k-steps in a qkp PSUM tile.
    def v_chain(m, c0, cw):
        # uniform [128,512] qkp tiles (bank-exclusive) even for the 256-col
        # chain, so start=True zero-regions never alias a neighbor
        ps_full = qkp.tile([P, N // 2], F32, tag="qk", name="vps")
        ps = ps_full[:, 0:cw]
        for k in range(DC):

            def job(k=k, ps=ps):
                nc.tensor.matmul(
                    ps,
                    lhsT=xT(k)[:, m * P : (m + 1) * P],
                    rhs=wq[k][:, 2 * D + c0 : 2 * D + c0 + cw],
                    start=(k == 0),
                    stop=(k == DC - 1),
                    skip_group_check=True,
                )

            yield ("mm", job)

        def finish(ps=ps):
            vp3 = vpad[m].rearrange("p (h c) -> p h c", c=HP)
            h0 = c0 // HD
            nc.vector.tensor_copy(
                vp3[:, h0 : h0 + cw // HD, 0:HD],
                ps.rearrange("p (h c) -> p h c", c=HD),
            )

        yield ("other", finish)

    def qkT_chain(t, half):
        ps = qkp.tile([P, N // 2], F32, tag="qk", name="qkps")
        for k in range(DC):

            def job(k=k, ps=ps):
                nc.tensor.matmul(
                    ps,
                    lhsT=wq[k][:, t * P : (t + 1) * P],
                    rhs=xT(k)[:, half * 512 : (half + 1) * 512],
                    start=(k == 0),
                    stop=(k == DC - 1),
                    skip_group_check=True,
                )

            yield ("mm", job)

        def finish(ps=ps):
            nc.vector.tensor_copy(qkT[t][:, half * 512 : (half + 1) * 512], ps)

        yield ("other", finish)

    # dense lead: pair-0 q/k half-0 (gates S(0)/exp(0) - the window start)
    # and v0 (PV chunk 0); v1-3 ride the queue thanks to the PV lag.
    for t in (6, 0):
        for kind, job in qkT_chain(t, 0):
            job()
    for c0, cw in ((0, 512), (512, 256)):
        for kind, job in v_chain(0, c0, cw):
            job()
    # bulk q/k rest columns k3-5 on the sync ring (free after x)
    for k in range(3, DC):
        nc.sync.dma_start(out=w32qkr[k][:, 0:640], in_=w_qkv[k * P : (k + 1) * P, P:D])
        nc.sync.dma_start(out=w32qkr[k][:, 640:1280], in_=w_qkv[k * P : (k + 1) * P, D + P : 2 * D])

    # work queue: (deadline_chunk, kind, thunk). Tile semantics are
    # sequential-program-order, so a chain MUST be emitted before the
    # chunk that consumes it: feed(t) drains every item with deadline
    # <= t+2 (covers the s(t+2)/o(t+1) emitted in iteration t+1), and
    # additionally up to `quota` matmuls per chunk to spread the rest
    # evenly through the window's PE slack.
    work = deque()
    mm_left = 0

    def push(deadline, gen):
        nonlocal mm_left
        for kind, job in gen:
            work.append((deadline, kind, job))
            if kind == "mm":
                mm_left += 1

    # PV emission lags exp/S by LAG chunks, so v-chain deadlines sit past
    # the S-critical qkT chains and the quota can spread them (deadline
    # bursts of v chains ahead of S supply starved exp for ~16us).
    LAG = 4

    def cast_job(dst, src):
        yield ("other", lambda: nc.vector.tensor_copy(dst, src))

    push(4, qkT_chain(6, 1))  # S(0,0,m>=4) reads k-tile cols 512:1024
    for m in (1, 2, 3):
        for c0, cw in ((0, 512), (512, 256)):
            push(m + LAG, v_chain(m, c0, cw))
    push(8, qkT_chain(0, 1))  # S(0,1,*) rhs
    for c0, cw in ((0, 512), (512, 256)):
        push(4 + LAG, v_chain(4, c0, cw))
    for c0, cw in ((0, 512), (512, 256)):
        push(5 + LAG, v_chain(5, c0, cw))
    for c0, cw in ((0, 512), (512, 256)):
        push(6 + LAG, v_chain(6, c0, cw))
    # deferred weight casts (DVE): q/k tiles 1-5,7-11 cols
    for k in (0, 1):
        push(10, cast_job(wq[k][:, P:D], w32qkr[k][:, 0:640]))
        push(10, cast_job(wq[k][:, D + P : 2 * D], w32qkr[k][:, 640:1280]))
    for c0, cw in ((0, 512), (512, 256)):
        push(7 + LAG, v_chain(7, c0, cw))
    for k in (2, 3):
        push(11, cast_job(wq[k][:, P:D], w32qkr[k][:, 0:640]))
        push(11, cast_job(wq[k][:, D + P : 2 * D], w32qkr[k][:, 640:1280]))
    for k in (4, 5):
        push(12, cast_job(wq[k][:, P:D], w32qkr[k][:, 0:640]))
        push(12, cast_job(wq[k][:, D + P : 2 * D], w32qkr[k][:, 640:1280]))
    # keep deadlines monotone: the drain in feed() only inspects the head
    for p in range(1, 6):
        push(16 * p, qkT_chain(6 + p, 0))
        push(16 * p, qkT_chain(p, 0))
        push(16 * p + 4, qkT_chain(6 + p, 1))
        push(16 * p + 8, qkT_chain(p, 1))
    for k in range(DC):
        push(88, cast_job(wp[k], w32p[k]))

    def feed(t, T):
        nonlocal mm_left
        rem = T - t
        quota = min(3, -(-mm_left // rem)) if rem > 0 else mm_left
        issued = 0
        while work:
            deadline, kind, job = work[0]
            if deadline > t + 2 and issued >= quota:
                break
            work.popleft()
            job()
            if kind == "mm":
                issued += 1
                mm_left -= 1

    # ---------------- attention window ----------------
    # chunks (p, n2, m): n2-OUTER so only one qpos-half's oaug pair is live.
    chunks = [(p, n2, m) for p in range(H // 2) for n2 in range(2) for m in range(NT)]
    T = len(chunks)
    sps = {}
    epool = {}
    oaug = {}
    osbs = {}

    def emit_s(t):
        p, n2, m = chunks[t]
        sp = mm.tile([P, N], F32, tag="mm", name="sps")
        sps[t] = sp
        for half in range(2):
            row = half * HD
            nc.tensor.matmul(
                sp[:, half * 512 : (half + 1) * 512],
                lhsT=qkT[6 + p][row : row + HD, m * P : (m + 1) * P],
                rhs=qkT[p][row : row + HD, n2 * 512 : (n2 + 1) * 512],
                start=True,
                stop=True,
            )

    def emit_exp(t):
        e = att.tile([P, N], F16, tag="e", name="etile", bufs=11)
        epool[t] = e
        nc.scalar.activation(e, sps.pop(t), exp, scale=SCALE)

    def emit_osb(h, n2, oa):
        # gpsimd cannot read PSUM; copies go to DVE (emitted ahead of the
        # feed so they run early and release the oaug slots promptly).
        # Both qpos halves of a head land in ONE [65,1024] tile so the
        # norm runs once per head.
        if n2 == 0:
            osbs[h] = att.tile([HP, N], F32, tag="osb", name="osb", bufs=3)
        nc.vector.tensor_copy(osbs[h][:, n2 * 512 : (n2 + 1) * 512], oa)

    z8s = {}

    def emit_zfetch(h, n2, oa):
        # partition-spread the Z row [1,512] -> [128,4]: bounce through
        # DRAM (only DRAM APs can remap free elements onto partitions);
        # launched at each half's end so it's off the tail's critical path
        if n2 == 0:
            z8s[h] = att.tile([P, 8], F32, tag="z8", name="z8", bufs=3)
        zd = zspill.tile([1, N // 2], F32, tag="zd", name="zd", bufs=3)
        nc.sync.dma_start(out=zd, in_=osbs[h][HD : HD + 1, n2 * 512 : (n2 + 1) * 512])
        nc.sync.dma_start(
            out=z8s[h][:, n2 * 4 : (n2 + 1) * 4],
            in_=zd.rearrange("o (p f) -> (o p) f", p=P),
        )

    def emit_norm(h, n2=None):
        # oT[h//2][row:row+64, cols] = osb rows 0:64 / Z  (Z = osb row 64).
        # reciprocal is ~6 cyc/ELEMENT serial per partition, so it runs on
        # the [128,4/8] partition-spread Z; 1/Z bounces through DRAM for
        # the partition-broadcast. n2=None: whole head (fewer DMAs);
        # n2 given: per qpos half (last pair, so only half-1 hits the tail).
        row = (h % 2) * HD
        if n2 is None:
            oX = osbs.pop(h)
            z8 = z8s.pop(h)
            r8 = att.tile([P, 8], F32, tag="r8", name="r8", bufs=2)
            nc.vector.reciprocal(r8, z8)
            rd = zspill.tile([1, N], F32, tag="rd", name="rd", bufs=2)
            nc.sync.dma_start(
                out=rd.rearrange("o (h p f) -> (o p) h f", h=2, f=4),
                in_=r8.rearrange("p (h f) -> p h f", f=4),
            )
            zb = att.tile([HD, N], F32, tag="zb", name="zbcast", bufs=2)
            nc.sync.dma_start(out=zb, in_=rd[0, :].partition_broadcast(HD))
            dst = oT[h // 2][row : row + HD, :]
            if h % 2 == 0:
                nc.vector.tensor_mul(dst, oX[0:HD, :], zb)
            else:
                nc.gpsimd.tensor_mul(dst, oX[0:HD, :], zb)
        else:
            oX = osbs[h] if n2 == 0 else osbs.pop(h)
            z8 = z8s[h] if n2 == 0 else z8s.pop(h)
            cols = slice(n2 * 512, (n2 + 1) * 512)
            r8 = att.tile([P, 8], F32, tag="r8", name="r8", bufs=2)[:, 0:4]
            nc.vector.reciprocal(r8, z8[:, n2 * 4 : (n2 + 1) * 4])
            rd = zspill.tile([1, N], F32, tag="rd", name="rd", bufs=2)[:, 0:512]
            nc.sync.dma_start(out=rd.rearrange("o (p f) -> (o p) f", p=P), in_=r8)
            zb = att.tile([HD, N], F32, tag="zb", name="zbcast", bufs=2)[:, 0:512]
            nc.sync.dma_start(out=zb, in_=rd[0, :].partition_broadcast(HD))
            dst = oT[h // 2][row : row + HD, cols]
            nc.vector.tensor_mul(dst, oX[0:HD, cols], zb)

    def emit_o(t):
        p, n2, m = chunks[t]
        e = epool.pop(t)
        for half in range(2):
            h = 2 * p + half
            if m == 0:
                oaug[h] = attps.tile([HP, N // 2], F32, tag="oaug", name="oaug", bufs=2)
            nc.tensor.matmul(
                oaug[h],
                lhsT=vpad[m][:, h * HP : (h + 1) * HP],
                rhs=e[:, half * 512 : (half + 1) * 512],
                start=(m == 0),
                stop=(m == NT - 1),
                skip_group_check=True,
            )
        if m == NT - 1:
            for half in range(2):
                h = 2 * p + half
                oa = oaug.pop(h)
                emit_osb(h, n2, oa)
                emit_zfetch(h, n2, oa)
            if p == H // 2 - 1:
                # last pair: per-half norms so only half-1 sits on the tail
                emit_norm(2 * p, n2)
                emit_norm(2 * p + 1, n2)
            elif n2 == 1:
                emit_norm(2 * p)
                emit_norm(2 * p + 1)

    emit_s(0)
    for t in range(T):
        emit_exp(t)
        if t + 1 < T:
            emit_s(t + 1)
        if t >= LAG:
            emit_o(t - LAG)
        feed(t, T)
    for t in range(T - LAG, T):
        emit_o(t)

    while work:
        _, _, job = work.popleft()
        job()

    # ---------------- proj drain: y = oT.T @ w_proj + b ----------------
    # 4 i-blocks in flight per round (2 full-width mm slots + col-split
    # chains on the freed qkp and attps slots), k-steps round-robin so the
    # k=5 steps (waiting on the last pair's norm) sit behind useful work.
    def proj_round(iblocks):
        pss = {}
        for g, i in enumerate(iblocks):
            if g < 2:
                ps = mm.tile([P, N], F32, tag="mm", name="pjps")
                pss[i] = (ps[:, 0:512], ps[:, 512:D], "mm")
            else:
                pool = qkp if g == 2 else attps
                tag = "qk" if g == 2 else "oaug"
                psA = pool.tile([P, 512], F32, tag=tag, name="pjA")
                psB = pool.tile([P, 512], F32, tag=tag, name="pjB")
                pss[i] = (psA, psB[:, 0:256], "cols")
        for k in range(DC):
            for i in iblocks:
                psA, psB, _ = pss[i]
                for ps_, c0, cw in ((psA, 0, 512), (psB, 512, 256)):
                    nc.tensor.matmul(
                        ps_,
                        lhsT=oT[k][:, i * P : (i + 1) * P],
                        rhs=wp[k][:, c0 : c0 + cw],
                        start=(k == 0),
                        stop=(k == DC - 1),
                        skip_group_check=True,
                    )
        for i in iblocks:
            psA, psB, kind = pss[i]
            yt = att.tile([P, D], F32, tag="yt", name="ytile", bufs=3)
            if kind == "mm":
                nc.vector.tensor_add(yt, psA.tensor[0:P, 0:D], brep)
            else:
                nc.vector.tensor_add(yt[:, 0:512], psA, brep[:, 0:512])
                nc.vector.tensor_add(yt[:, 512:D], psB, brep[:, 512:D])
            nc.sync.dma_start(out=y[i * P : (i + 1) * P, :], in_=yt)

    proj_round([0, 1, 2, 3])
    proj_round([4, 5, 6, 7])


def build_nc(debug: bool = False):
    nc = bacc.Bacc("TRN2", target_bir_lowering=False, debug=debug, enable_asserts=False)
    x = nc.dram_tensor("x", [N, D], F32, kind="ExternalInput").ap()
    w_qkv = nc.dram_tensor("w_qkv", [D, 3 * D], F32, kind="ExternalInput").ap()
    w_proj = nc.dram_tensor("w_proj", [D, D], F32, kind="ExternalInput").ap()
    b_proj = nc.dram_tensor("b_proj", [D], F32, kind="ExternalInput").ap()
    y = nc.dram_tensor("y", [N, D], F32, kind="ExternalOutput").ap()
    with tile.TileContext(nc) as tc:
        with ExitStack() as ctx:
            build_attention(ctx, tc, x, w_qkv, w_proj, b_proj, y)
    nc.compile()
    return nc


_NC = None


def _get_nc():
    global _NC
    if _NC is None:
        _NC = build_nc()
    return _NC


def kernel(inputs, w_qkv, w_proj, b_proj, _trace=False, **run_kwargs):
    from concourse.bass_utils import run_bass_kernel_spmd

    nc = _get_nc()
    inputs = np.asarray(inputs, dtype=np.float32)
    w_qkv = np.ascontiguousarray(np.asarray(w_qkv, dtype=np.float32))
    w_proj = np.ascontiguousarray(np.asarray(w_proj, dtype=np.float32))
    b_proj = np.ascontiguousarray(np.asarray(b_proj, dtype=np.float32))
    in_maps = [
        {
            "x": np.ascontiguousarray(inputs[i]),
            "w_qkv": w_qkv,
            "w_proj": w_proj,
            "b_proj": b_proj,
        }
        for i in range(NCORES)
    ]
    res = run_bass_kernel_spmd(nc, in_maps, list(range(NCORES)), trace=_trace, **run_kwargs)
    out = np.stack([res.results[i]["y"] for i in range(NCORES)], axis=0)
    if _trace:
        return out, res
    return out
